# revision 1
# baseline (speedup 1.0000x reference)
"""Trainium2 Bass kernel for ContinuousDGM message passing.

  xe = x @ W_emb + b_emb            [N, E]
  D  = sq_cdist(xe)                 [N, N]
  A  = 1 / (1 + D)
  W  = A / A.sum(axis=1)            (broadcast over last axis -> col-normalize)
  out = W @ xe                      [N, E]

Strategy (8 NeuronCores, row-block sharding, fully fused -- the [N,N]
matrices never touch DRAM):
  * Host passes x transposed AND pre-cast to bf16 (xT [DIN, N] bf16) plus
    the core's own column block (xTl), so every matmul runs at bf16 rate
    with contraction on partitions.
  * Augmented operands: augL [68, N] = [-2*xeT; ones; ones; sq_hi; sq_lo]
    (stationary), augRl [68, B] = [xeT_local; (sq+1) hi/lo; ones; ones]
    (moving). One matmul per j-tile produces psum [128, 1024] = 1 + D.
    sq rides as a bf16 hi+lo pair computed from the same bf16 xe values
    the PE multiplies, so the diagonal cancels without masking.
  * A = reciprocal(psum) on DVE (approx_fast); ACT casts f32->bf16 into
    the SBUF stash AND accumulates per-partition row sums r[j] =
    sum_i A[j, i_local] for free (accum_out).
  * s[j] = sum over cores of r[j] via three chunked AllGathers + local
    adds; ye = xe * (1/s).  Chunk collectives overlap the A pass; the
    final chunk is small so its collective barely extends the tail.
    Out^T matmuls for early tiles interleave with late G matmuls.
  * out^T[e, i] accumulates in one PSUM region over all 64 j-tiles.
  * Host concatenates the 8 out^T blocks and transposes.
"""

import os
import sys

import numpy as np

N, DIN, E = 8192, 256, 64
P = 128
C = 8
B = N // C            # 1024 rows per core
SUP = 512
NSUP = N // SUP       # 16
BSUP = B // SUP       # 2
NT = N // P           # 64
BT = B // P           # 8
CH = [(0, 32), (32, 48), (48, 64)]   # r-chunk (start, end) j-tiles
ILV = 48              # j-tile index where out-matmul interleave starts
MERGED = False        # 1024-col matmuls rejected: ISA caps f32 out at 512

_NC_CACHE = {}


def _import_concourse():
    try:
        import concourse.bacc  # noqa: F401
    except ImportError:
        for p in ("/opt/trn_rl_repo", "/root/.axon_site/_ro/trn_rl_repo"):
            if os.path.isdir(p) and p not in sys.path:
                sys.path.insert(0, p)
        import concourse.bacc  # noqa: F401


def build_body(tc, outT, xT, xTl, W, b, eye):
    """Emit the kernel body. All args are bass APs of DRAM tensors."""
    from contextlib import ExitStack

    import concourse.bass as bass  # noqa: F401
    from concourse import mybir

    nc = tc.nc
    f32 = mybir.dt.float32
    bf16 = mybir.dt.bfloat16
    AF = mybir.ActivationFunctionType
    ALU = mybir.AluOpType

    with ExitStack() as ctx:
        big = ctx.enter_context(tc.tile_pool(name="big", bufs=1))
        const = ctx.enter_context(tc.tile_pool(name="const", bufs=1))
        work = ctx.enter_context(tc.tile_pool(name="work", bufs=1))
        psum = ctx.enter_context(tc.tile_pool(name="psum", bufs=1, space="PSUM"))
        dram = ctx.enter_context(tc.tile_pool(name="dram", bufs=1, space="DRAM"))

        # ---------- load constants ----------
        Wsb = const.tile([P, 2, E], bf16, name="Wsb", tag="Wsb")
        for t in range(2):
            nc.sync.dma_start(Wsb[:, t, :], W[t * P:(t + 1) * P, :])
        b_col = const.tile([E, 1], f32, name="bcol", tag="bcol")
        nc.sync.dma_start(b_col[:], b[:])
        b2_col = const.tile([E, 1], f32, name="b2col", tag="b2col")
        nc.vector.tensor_scalar_mul(b2_col[:], b_col[:], -2.0)
        eye_f = const.tile([P, P], f32, name="eyef", tag="eyef")
        nc.sync.dma_start(eye_f[:], eye[:])
        eye_b = const.tile([P, P], bf16, name="eyeb", tag="eyeb")
        nc.scalar.copy(eye_b[:], eye_f[:])

        # ---------- operand buffers ----------
        # augL: [0:64]=-2*xeT, [64:66]=1, [66:68]=sq hi/lo   (stationary)
        # augRl: [0:64]=xeT local, [64:66]=(sq+1) hi/lo, [66:68]=1 (moving)
        # There is no separate full xe buffer: -2*xe holds the same values
        # up to an exact power-of-two factor, so the row-major transposes
        # read augL and rescale by -0.5 (exact).
        augL = big.tile([68, N], bf16, name="augL", tag="augL")
        augRl = big.tile([68, B], bf16, name="augRl", tag="augRl")
        nc.vector.memset(augL[64:66, :], 1.0)
        # partition start 66 is not engine-addressable; stage ones via DMA.
        onesrow = work.tile([2, B], bf16, name="onesrow", tag="onesrow")
        nc.vector.memset(onesrow[:], 1.0)
        nc.sync.dma_start(augRl[66:68, :], onesrow[:])

        # xeT supers: psum[e, i] = sum_k W[k, e] * x[i, k]  (+ b via ACT bias)
        # x chunks stream from DRAM on two DMA queues (sync + scalar).
        def emit_xeT(xsrc, s, dst, lneg):
            ps = psum.tile([E, SUP], f32, name="ps", tag="ps", bufs=2)
            for t in range(2):
                xc = work.tile([P, SUP], bf16, name="xc", tag="xc", bufs=4)
                nc.sync.dma_start(
                    xc[:], xsrc[t * P:(t + 1) * P, s * SUP:(s + 1) * SUP])
                nc.tensor.matmul(
                    ps[:], lhsT=Wsb[:, t, :], rhs=xc[:],
                    start=(t == 0), stop=(t == 1),
                )
            sl = slice(s * SUP, (s + 1) * SUP)
            if lneg:
                nc.scalar.activation(dst[0:64, sl], ps[:], AF.Identity,
                                     bias=b2_col[:], scale=-2.0)
            else:
                nc.scalar.activation(dst[0:64, sl], ps[:], AF.Identity,
                                     bias=b_col[:], scale=1.0)

        for s in range(NSUP):
            emit_xeT(xT, s, augL, True)
        for s in range(BSUP):
            emit_xeT(xTl, s, augRl, False)

        # ---------- row-major bf16 xe + sq (from the SAME bf16 values) ----------
        # squares alternate ACT / DVE to balance the two engines
        xe_bf = big.tile([P, NT * E], bf16, name="xebf", tag="xebf")
        sq_mat = const.tile([P, NT], f32, name="sqmat", tag="sqmat")


        for it in range(NT):
            pt = psum.tile([P, E], bf16, name="pt", tag="ps", bufs=2)
            nc.tensor.transpose(pt[:], augL[0:64, it * P:(it + 1) * P],
                                eye_b[0:64, 0:64])
            xs = xe_bf[:, it * E:(it + 1) * E]
            nc.vector.tensor_scalar_mul(xs, pt[:], -0.5)
            junkE = work.tile([P, E], bf16, name="junkE", tag="junkE", bufs=2)
            # ACT squares the psum tile (-2xe): scale -0.5 is exact
            nc.scalar.activation(junkE[:], pt[:], AF.Square, scale=-0.5,
                                 accum_out=sq_mat[:, it:it + 1])
        sql_mat = const.tile([P, BT], f32, name="sqlmat", tag="sqlmat")
        for it in range(BT):
            pt = psum.tile([P, E], bf16, name="pt", tag="ps", bufs=2)
            nc.tensor.transpose(pt[:], augRl[0:64, it * P:(it + 1) * P],
                                eye_b[0:64, 0:64])
            junkE = work.tile([P, E], bf16, name="junkE", tag="junkE", bufs=2)
            nc.scalar.activation(junkE[:], pt[:], AF.Square,
                                 accum_out=sql_mat[:, it:it + 1])

        # ---------- sq rows (hi/lo bf16) -> aug rows ----------
        def sq_rows(sq_tile_, nt, plus_one, dst0, dst1):
            # sq [128, nt] -> T [nt, 128] -> hi/lo splits -> DMA into rows
            pt2 = psum.tile([nt, P], f32, name="pt2", tag="ps", bufs=2)
            nc.tensor.transpose(pt2[:], sq_tile_[:], eye_f[:])
            T = work.tile([nt, P], f32, name="Tf", tag="Tf", bufs=2)
            if plus_one:
                nc.vector.tensor_scalar_add(T[:], pt2[:], 1.0)
            else:
                nc.scalar.copy(T[:], pt2[:])
            hi = work.tile([nt, P], bf16, name="hi", tag="hi", bufs=2)
            nc.scalar.copy(hi[:], T[:])
            hif = work.tile([nt, P], f32, name="hif", tag="hif", bufs=2)
            nc.vector.tensor_copy(out=hif[:], in_=hi[:])
            lo = work.tile([nt, P], f32, name="lo", tag="lo", bufs=2)
            nc.vector.tensor_tensor(lo[:], T[:], hif[:], ALU.subtract)
            lob = work.tile([nt, P], bf16, name="lob", tag="lob", bufs=2)
            nc.scalar.copy(lob[:], lo[:])
            nc.sync.dma_start(dst0, hi[:])
            nc.sync.dma_start(dst1, lob[:])

        sq_rows(sq_mat, NT, False, augL[66:67, 0:N], augL[67:68, 0:N])
        sq_rows(sql_mat, BT, True, augRl[64:65, 0:B], augRl[65:66, 0:B])

        # ---------- A pass + fused row sums + out^T accumulation ----------
        r_mat = const.tile([P, NT], f32, name="rmat", tag="rmat")
        atb = [big.tile([P, B], bf16, name=f"atb{jt}", tag=f"atb{jt}")
               for jt in range(NT)]
        po = psum.tile([E, B], f32, name="po", tag="po", bufs=1)

        agin, agout, rg, rs = [], [], [], []
        for h, (c0, c1) in enumerate(CH):
            w = c1 - c0
            agin.append(dram.tile([P * w], f32, name=f"agin{h}",
                                  tag=f"agin{h}"))
            agout.append(dram.tile([C * P * w], f32, name=f"agout{h}",
                                   tag=f"agout{h}", addr_space="Shared"))
            rg.append(const.tile([P, C * w], f32, name=f"rg{h}", tag=f"rg{h}"))
            rs.append(const.tile([P, w], f32, name=f"rs{h}", tag=f"rs{h}"))

        def g_tile(jt):
            pg = psum.tile([P, B], f32, name="pg", tag="pg", bufs=2)
            if MERGED:
                nc.tensor.matmul(pg[:], lhsT=augL[:, jt * P:(jt + 1) * P],
                                 rhs=augRl[:, :], start=True, stop=True)
            else:
                for h in range(BSUP):
                    nc.tensor.matmul(pg[:, h * SUP:(h + 1) * SUP],
                                     lhsT=augL[:, jt * P:(jt + 1) * P],
                                     rhs=augRl[:, h * SUP:(h + 1) * SUP],
                                     start=True, stop=True)
            ar = work.tile([P, B], f32, name="ar", tag="ar", bufs=2)
            nc.vector.reciprocal_approx_fast(out=ar[:], in_=pg[:])
            nc.scalar.activation(atb[jt][:], ar[:], AF.Identity,
                                 accum_out=r_mat[:, jt:jt + 1])

        def out_tile(k):
            if MERGED:
                nc.tensor.matmul(po[:], lhsT=xe_bf[:, k * E:(k + 1) * E],
                                 rhs=atb[k][:], start=(k == 0),
                                 stop=(k == NT - 1))
            else:
                for h in range(BSUP):
                    nc.tensor.matmul(po[:, h * SUP:(h + 1) * SUP],
                                     lhsT=xe_bf[:, k * E:(k + 1) * E],
                                     rhs=atb[k][:, h * SUP:(h + 1) * SUP],
                                     start=(k == 0), stop=(k == NT - 1))

        def flush_chunk(h):
            c0, c1 = CH[h]
            nc.sync.dma_start(agin[h][:], r_mat[:, c0:c1])
            nc.gpsimd.collective_compute(
                "AllGather", ALU.bypass,
                replica_groups=[list(range(C))],
                ins=[agin[h][:]], outs=[agout[h][:]],
            )
            w = c1 - c0
            for c in range(C):
                nc.sync.dma_start(rg[h][:, c * w:(c + 1) * w],
                                  agout[h][c * P * w:(c + 1) * P * w])

        def reduce_chunk(h):
            # s = sum of the 8 per-core partials; rs = 1/s  (DVE)
            c0, c1 = CH[h]
            w = c1 - c0
            s0 = work.tile([P, w], f32, name=f"s{h}", tag=f"s{h}", bufs=1)
            nc.vector.tensor_tensor(s0[:], rg[h][:, 0:w], rg[h][:, w:2 * w],
                                    ALU.add)
            for c in range(2, C):
                nc.vector.tensor_tensor(
                    s0[:], s0[:], rg[h][:, c * w:(c + 1) * w], ALU.add)
            nc.vector.reciprocal(rs[h][:], s0[:])

        def ye_range(h, on_act=False):
            c0, c1 = CH[h]
            for t in range(c1 - c0):
                jt = c0 + t
                sl = slice(jt * E, (jt + 1) * E)
                if on_act:
                    nc.scalar.activation(xe_bf[:, sl], xe_bf[:, sl],
                                         AF.Identity, scale=rs[h][:, t:t + 1])
                else:
                    nc.vector.tensor_scalar_mul(xe_bf[:, sl], xe_bf[:, sl],
                                                rs[h][:, t:t + 1])

        for jt in range(0, 32):
            g_tile(jt)
        flush_chunk(0)
        for jt in range(32, ILV):
            g_tile(jt)
        reduce_chunk(0)     # DVE: AllGather 0 has landed by now
        ye_range(0)         # DVE
        flush_chunk(1)
        no = 0
        for jt in range(ILV, NT):
            g_tile(jt)
            out_tile(no)
            no += 1
        flush_chunk(2)
        reduce_chunk(1)
        ye_range(1)
        for k in range(no, CH[2][0]):
            out_tile(k)
        reduce_chunk(2)
        ye_range(2)
        for k in range(CH[2][0], NT):
            out_tile(k)

        osb = work.tile([E, B], f32, name="osb", tag="osb", bufs=1)
        nc.scalar.copy(osb[:], po[:])
        nc.sync.dma_start(outT[:, :], osb[:])


def _build_nc():
    _import_concourse()
    import concourse.bacc as bacc
    import concourse.tile as tile
    from concourse import mybir

    f32 = mybir.dt.float32
    bf16 = mybir.dt.bfloat16
    nc = bacc.Bacc("TRN2", target_bir_lowering=False, debug=False,
                   num_devices=C)
    xT = nc.dram_tensor("xT", [DIN, N], bf16, kind="ExternalInput").ap()
    xTl = nc.dram_tensor("xTl", [DIN, B], bf16, kind="ExternalInput").ap()
    W = nc.dram_tensor("W", [DIN, E], bf16, kind="ExternalInput").ap()
    b = nc.dram_tensor("b", [E, 1], f32, kind="ExternalInput").ap()
    eye = nc.dram_tensor("eye", [P, P], f32, kind="ExternalInput").ap()
    outT = nc.dram_tensor("outT", [E, B], f32, kind="ExternalOutput").ap()

    with tile.TileContext(nc) as tc:
        build_body(tc, outT, xT, xTl, W, b, eye)
    nc.compile()
    return nc


def make_in_maps(x, W_emb, b_emb):
    import ml_dtypes

    bf = ml_dtypes.bfloat16
    xT = np.ascontiguousarray(x.T).astype(bf)
    eye = np.eye(P, dtype=np.float32)
    bb = np.asarray(b_emb, dtype=np.float32).reshape(E, 1)
    Wf = np.asarray(W_emb, dtype=np.float32).astype(bf)
    in_maps = []
    for c in range(C):
        in_maps.append({
            "xT": xT,
            "xTl": np.ascontiguousarray(xT[:, c * B:(c + 1) * B]),
            "W": Wf,
            "b": bb,
            "eye": eye,
        })
    return in_maps


def kernel(x, W_emb, b_emb, _trace=False, _tmpdir=None):
    _import_concourse()
    from concourse import bass_utils

    key = "nc"
    if key not in _NC_CACHE:
        _NC_CACHE[key] = _build_nc()
    nc = _NC_CACHE[key]

    in_maps = make_in_maps(np.asarray(x), np.asarray(W_emb), np.asarray(b_emb))
    res = bass_utils.run_bass_kernel_spmd(
        nc, in_maps, core_ids=list(range(C)),
        trace=_trace, tmpdir=_tmpdir,
    )
    blocks = [np.asarray(res.results[c]["outT"]) for c in range(C)]
    outT = np.concatenate(blocks, axis=1)          # [E, N]
    out = np.ascontiguousarray(outT.T).astype(np.float32)  # [N, E]
    if _trace:
        return out, res
    return out



# revision 12
# speedup vs baseline: 1.2796x; 1.2796x over previous
"""Trainium2 Bass kernel for ContinuousDGM message passing.

  xe = x @ W_emb + b_emb            [N, E]
  D  = sq_cdist(xe)                 [N, N]
  A  = 1 / (1 + D)
  W  = A / A.sum(axis=1)            (broadcast over last axis -> col-normalize)
  out = W @ xe                      [N, E]

v2 strategy (8 NeuronCores, row-block sharding, fully fused; [N,N] never
touches DRAM). Per core: columns block of 1024 (all 8192 rows).

  * Pre-G: xe^T supers via PE (W stationary, x chunks moving, 2 DMA
    queues); PE transposes build row-major xe_bf in 16-tile batches in
    one PSUM bank; squares batched (ACT Square + DVE grouped reduce);
    sq rides as ONE bf16 aug row (no hi/lo - diagonal error ~bf16 sq
    precision, acceptable at 2e-2 tolerance).
  * A-pass per j-tile: 2 G matmuls -> psum[128,1024] f32 = 1+D.
    Reciprocal split across engines: ~2/3 tiles on ACT via raw
    InstActivation Reciprocal (table is bf16-accurate; one pass does
    recip + bf16 cast + row-sum accum), ~1/3 on DVE via
    reciprocal_approx_fast + tensor_tensor_reduce (cast + row-sum).
  * Row-sum partials r[j] all-reduced in 4 chunks of 16 j-tiles
    (gpsimd AllReduce, overlapped with later A-pass work); ye = xe/s
    applied in-place per chunk via one broadcast tensor_tensor.
  * Out pass runs after the A-pass with ACT/DVE idle so HAM un-throttles
    the PE to 2.4 GHz: 128 accumulating matmuls into po[64,1024].
  * outT DMA'd straight from PSUM; host concatenates and transposes.
"""

import os
import sys

import numpy as np

N, DIN, E = 8192, 256, 64
P = 128
C = 8
B = N // C            # 1024 cols per core
SUP = 512
NSUP = N // SUP       # 16
BSUP = B // SUP       # 2
NT = N // P           # 64
BT = B // P           # 8
AUG = 68              # 64 xe rows + ones + sq hi/lo (diag must cancel ~1e-3)
GRP = 16              # j-tiles per pre-G batch / per collective chunk
NG = NT // GRP        # 4 groups/chunks

_NC_CACHE = {}
DEBUG_DUMPS = False


def _import_concourse():
    try:
        import concourse.bacc  # noqa: F401
    except ImportError:
        for p in ("/opt/trn_rl_repo", "/root/.axon_site/_ro/trn_rl_repo"):
            if os.path.isdir(p) and p not in sys.path:
                sys.path.insert(0, p)
        import concourse.bacc  # noqa: F401


def _act_raw(nc, out, in_, func, accum_out=None, scale=1.0, bias=0.0):
    """Emit InstActivation directly (the helper blocks Reciprocal; its
    table is accurate to ~4e-3 which is fine at our 2e-2 tolerance)."""
    from concourse import mybir

    eng = nc.scalar
    inputs = [eng.lower_ap(in_)]
    for arg in (bias, scale, 0.0):
        if isinstance(arg, float):
            inputs.append(mybir.ImmediateValue(dtype=mybir.dt.float32, value=arg))
        else:
            inputs.append(eng.lower_ap(arg))
    outputs = [eng.lower_ap(out)]
    if accum_out is not None:
        outputs.append(eng.lower_ap(accum_out))
    return eng.add_instruction(mybir.InstActivation(
        name=nc.get_next_instruction_name(),
        func=func, ins=inputs, outs=outputs))


def build_body(tc, outT, xT, xTl, W, b, eye, dbg_aps=None):
    from contextlib import ExitStack

    if DEBUG_DUMPS:
        (dbg_sq, dbg_r, dbg_s, dbg_a0, dbg_a2, dbg_xe, dbg_augL,
         dbg_augR) = dbg_aps

    from concourse import mybir
    from concourse.bass import broadcast_tensor_aps

    nc = tc.nc
    f32 = mybir.dt.float32
    bf16 = mybir.dt.bfloat16
    AF = mybir.ActivationFunctionType
    ALU = mybir.AluOpType

    with ExitStack() as ctx:
        big = ctx.enter_context(tc.tile_pool(name="big", bufs=1))
        const = ctx.enter_context(tc.tile_pool(name="const", bufs=1))
        work = ctx.enter_context(tc.tile_pool(name="work", bufs=1))
        dram = ctx.enter_context(tc.tile_pool(name="dram", bufs=1, space="DRAM"))

        # ---------- constants ----------
        Wsb = const.tile([P, 2, E], bf16, name="Wsb", tag="Wsb")
        for t in range(2):
            nc.sync.dma_start(Wsb[:, t, :], W[t * P:(t + 1) * P, :])
        b_col = const.tile([E, 1], f32, name="bcol", tag="bcol")
        nc.sync.dma_start(b_col[:], b[:])
        b2_col = const.tile([E, 1], f32, name="b2col", tag="b2col")
        nc.vector.tensor_scalar_mul(b2_col[:], b_col[:], -2.0)
        eye_f = const.tile([P, P], f32, name="eyef", tag="eyef")
        nc.sync.dma_start(eye_f[:], eye[:])
        eye_b = const.tile([P, P], bf16, name="eyeb", tag="eyeb")
        nc.scalar.copy(eye_b[:], eye_f[:])

        # ---------- operand buffers ----------
        # augL: [0:64]=-2*xeT, [64:66]=ones, [66]=sq_hi, [67]=sq_lo
        # augRl: [0:64]=xeT local, [64]=(1+sq)hi, [65]=(1+sq)lo, [66:68]=ones
        # The sq hi/lo bf16 pair reproduces the f32 row norm to ~2^-16 so
        # the diagonal of 1+D cancels to ~1e-3 (out[i] ~ ye[i] + random-sign
        # noise of similar size, so the diagonal weight needs this).
        augL = big.tile([AUG, N], bf16, name="augL", tag="augL")
        augRl = big.tile([AUG, B], bf16, name="augRl", tag="augRl")
        # ones rows: engines can only address partition starts 0/32/64/96;
        # stage small memset tiles and DMA into rows >=64.
        ones_st = work.tile([64, 2 * P], bf16, name="ones_st", tag="ones_st")
        nc.vector.memset(ones_st[:], 1.0)
        nc.sync.dma_start(augL[64:66, :], ones_st[:, :])
        nc.sync.dma_start(augRl[66:68, :], ones_st[0:16, 0:P])

        xe_bf = big.tile([P, NT, E], bf16, name="xebf", tag="xebf")
        sq_mat = const.tile([P, NT], f32, name="sqmat", tag="sqmat")
        sql_mat = const.tile([P, BT], f32, name="sqlmat", tag="sqlmat")
        r_mat = const.tile([P, NT], f32, name="rmat", tag="rmat")
        junk = work.tile([P, GRP, E], f32, name="junk", tag="junk", bufs=2)
        srow_hi = work.tile([GRP, P], bf16, name="srow_hi", tag="srow_hi",
                            bufs=2)
        srow_lo = work.tile([GRP, P], bf16, name="srow_lo", tag="srow_lo",
                            bufs=2)
        hif = work.tile([GRP, P], f32, name="hif", tag="hif", bufs=2)
        lof = work.tile([GRP, P], f32, name="lof", tag="lof", bufs=2)

        atb = [big.tile([P, B], bf16, name=f"atb{jt}", tag=f"atb{jt}")
               for jt in range(NT)]

        agin, agout, s_sb, rs = [], [], [], []
        for h in range(NG):
            agin.append(dram.tile([P * GRP], f32, name=f"agin{h}",
                                  tag=f"agin{h}"))
            agout.append(dram.tile([P * GRP], f32, name=f"agout{h}",
                                   tag=f"agout{h}", addr_space="Shared"))
            s_sb.append(const.tile([P, GRP, 1], f32, name=f"ssb{h}",
                                   tag=f"ssb{h}"))
            rs.append(const.tile([P, GRP, 1], f32, name=f"rs{h}", tag=f"rs{h}"))

        # ================= pre-G phase =================
        with tc.tile_pool(name="psum_pre", bufs=1, space="PSUM") as psum_pre:

            def emit_xeT(xsrc, s, dst, lneg, qi):
                ps = psum_pre.tile([E, SUP], f32, name="ps", tag="ps", bufs=2)
                for t in range(2):
                    xc = work.tile([P, SUP], bf16, name="xc", tag="xc", bufs=6)
                    q = nc.sync if (qi + t) % 2 == 0 else nc.gpsimd
                    q.dma_start(
                        xc[:], xsrc[t * P:(t + 1) * P, s * SUP:(s + 1) * SUP])
                    nc.tensor.matmul(
                        ps[:], lhsT=Wsb[:, t, :], rhs=xc[:],
                        start=(t == 0), stop=(t == 1),
                    )
                sl = slice(s * SUP, (s + 1) * SUP)
                if lneg:
                    nc.scalar.activation(dst[0:64, sl], ps[:], AF.Identity,
                                         bias=b2_col[:], scale=-2.0)
                else:
                    nc.scalar.activation(dst[0:64, sl], ps[:], AF.Identity,
                                         bias=b_col[:], scale=1.0)

            ptg = psum_pre.tile([P, GRP, E], bf16, name="ptg", tag="ptg",
                                bufs=1)
            psq = psum_pre.tile([GRP, P], f32, name="psq", tag="psq", bufs=2)

            def hi_lo_rows(nt, plus_one, dst_hi, dst_lo):
                """psq[0:nt] (f32 sq values) -> bf16 hi/lo rows + DMA out."""
                if plus_one:
                    nc.scalar.activation(srow_hi[0:nt, :], psq[0:nt, :],
                                         AF.Identity, bias=1.0)
                else:
                    nc.scalar.copy(srow_hi[0:nt, :], psq[0:nt, :])
                nc.vector.tensor_copy(out=hif[0:nt, :], in_=srow_hi[0:nt, :])
                if plus_one:
                    nc.vector.tensor_scalar_add(lof[0:nt, :], psq[0:nt, :],
                                                1.0)
                    nc.vector.tensor_tensor(lof[0:nt, :], lof[0:nt, :],
                                            hif[0:nt, :], ALU.subtract)
                else:
                    nc.vector.tensor_tensor(lof[0:nt, :], psq[0:nt, :],
                                            hif[0:nt, :], ALU.subtract)
                nc.vector.tensor_copy(out=srow_lo[0:nt, :], in_=lof[0:nt, :])
                nc.sync.dma_start(dst_hi, srow_hi[0:nt, :])
                nc.sync.dma_start(dst_lo, srow_lo[0:nt, :])

            # ---- local block: augRl + sq_local ----
            for s in range(BSUP):
                emit_xeT(xTl, s, augRl, False, s)
            for it in range(BT):
                nc.tensor.transpose(ptg[:, it, :],
                                    augRl[0:64, it * P:(it + 1) * P],
                                    eye_b[0:64, 0:64])
            # squares of local xe: ptg holds xe (bf16) directly
            nc.scalar.activation(junk[:, 0:BT, :], ptg[:, 0:BT, :], AF.Square)
            nc.vector.tensor_reduce(out=sql_mat[:], in_=junk[:, 0:BT, :],
                                    axis=mybir.AxisListType.X, op=ALU.add)
            nc.tensor.transpose(psq[0:BT, :], sql_mat[:], eye_f[:])
            hi_lo_rows(BT, True, augRl[64:65, :], augRl[65:66, :])

            # ---- full rows: augL, xe_bf, sq row, in 4 groups ----
            for g in range(NG):
                for si in range(4):
                    s = g * 4 + si
                    emit_xeT(xT, s, augL, True, s)
                    for ti in range(4):
                        it = s * 4 + ti
                        nc.tensor.transpose(
                            ptg[:, it - g * GRP, :],
                            augL[0:64, it * P:(it + 1) * P],
                            eye_b[0:64, 0:64])
                gsl = slice(g * GRP, (g + 1) * GRP)
                # xe_bf = -0.5 * (-2 xe)  (exact)
                nc.vector.tensor_scalar_mul(xe_bf[:, gsl, :], ptg[:, :, :],
                                            -0.5)
                nc.scalar.activation(junk[:, :, :], xe_bf[:, gsl, :],
                                     AF.Square)
                nc.vector.tensor_reduce(out=sq_mat[:, gsl], in_=junk[:, :, :],
                                        axis=mybir.AxisListType.X, op=ALU.add)
                nc.tensor.transpose(psq[:, :], sq_mat[:, gsl], eye_f[:])
                gp = slice(g * GRP * P, (g + 1) * GRP * P)
                hi_lo_rows(GRP, False, augL[66:67, gp], augL[67:68, gp])

        # ================= A-pass + out =================
        with tc.tile_pool(name="psum_a", bufs=1, space="PSUM") as psum_a:
            po = psum_a.tile([E, B], f32, name="po", tag="po", bufs=1)

            def flush(h):
                nc.sync.dma_start(agin[h][:],
                                  r_mat[:, h * GRP:(h + 1) * GRP])
                nc.gpsimd.collective_compute(
                    "AllReduce", ALU.add,
                    replica_groups=[list(range(C))],
                    ins=[agin[h][:]], outs=[agout[h][:]],
                )
                nc.sync.dma_start(s_sb[h][:, :, :], agout[h][:])

            def ye(h):
                nc.vector.reciprocal(rs[h][:, :, :], s_sb[h][:, :, :])
                sl = slice(h * GRP, (h + 1) * GRP)
                b0, b1 = broadcast_tensor_aps(xe_bf[:, sl, :],
                                              rs[h][:, :, 0:1])
                nc.vector.tensor_tensor(xe_bf[:, sl, :], b0, b1, ALU.mult)

            for jt in range(NT):
                pg = psum_a.tile([P, B], f32, name="pg", tag="pg", bufs=2)
                for h in range(BSUP):
                    nc.tensor.matmul(pg[:, h * SUP:(h + 1) * SUP],
                                     lhsT=augL[:, jt * P:(jt + 1) * P],
                                     rhs=augRl[:, h * SUP:(h + 1) * SUP],
                                     start=True, stop=True)
                if jt % 5 in (1, 3):
                    # DVE lane: recip, then cast + row-sum in one
                    # tensor_scalar (2x_2p; accum needs op1)
                    ar = work.tile([P, B], f32, name="ar", tag="ar", bufs=2)
                    nc.vector.reciprocal_approx_fast(out=ar[:], in_=pg[:])
                    nc.vector.tensor_scalar(
                        out=atb[jt][:], in0=ar[:], scalar1=1.0, scalar2=None,
                        op0=ALU.mult, op1=ALU.add,
                        accum_out=r_mat[:, jt:jt + 1])
                else:
                    # ACT lane: one pass does recip + bf16 cast + row-sum
                    _act_raw(nc, atb[jt][:], pg[:], AF.Reciprocal,
                             accum_out=r_mat[:, jt:jt + 1])
                if jt % GRP == GRP - 1:
                    flush(jt // GRP)
                    if jt // GRP >= 1:
                        ye(jt // GRP - 1)

            # ---- out pass (chunks 0..2 first; chunk 3 waits its AllReduce)
            def out_tile(k):
                for h in range(BSUP):
                    nc.tensor.matmul(po[:, h * SUP:(h + 1) * SUP],
                                     lhsT=xe_bf[:, k, :],
                                     rhs=atb[k][:, h * SUP:(h + 1) * SUP],
                                     start=(k == 0), stop=(k == NT - 1))

            for k in range(0, 3 * GRP):
                out_tile(k)
            ye(3)
            for k in range(3 * GRP, NT):
                out_tile(k)

            osb = work.tile([E, B], f32, name="osb", tag="osb")
            nc.scalar.copy(osb[:], po[:])
            nc.sync.dma_start(outT[:, :], osb[:])

            if DEBUG_DUMPS:
                dbg = tc.nc  # alias
                dbg.sync.dma_start(dbg_sq[:, :], sq_mat[:, :])
                dbg.sync.dma_start(dbg_r[:, :], r_mat[:, :])
                for h in range(NG):
                    dbg.sync.dma_start(
                        dbg_s[:, h * GRP:(h + 1) * GRP], s_sb[h][:, :, 0])
                dbg.sync.dma_start(dbg_a0[:, :], atb[0][:])
                dbg.sync.dma_start(dbg_a2[:, :], atb[2][:])
                dbg.sync.dma_start(dbg_xe[:, :, :], xe_bf[:, :, :])
                dbg.sync.dma_start(dbg_augL[:, :], augL[:, :])
                dbg.sync.dma_start(dbg_augR[:, :], augRl[:, :])


def _build_nc():
    _import_concourse()
    import concourse.bacc as bacc
    import concourse.tile as tile
    from concourse import mybir

    f32 = mybir.dt.float32
    bf16 = mybir.dt.bfloat16
    nc = bacc.Bacc("TRN2", target_bir_lowering=False, debug=False,
                   num_devices=C)
    xT = nc.dram_tensor("xT", [DIN, N], bf16, kind="ExternalInput").ap()
    xTl = nc.dram_tensor("xTl", [DIN, B], bf16, kind="ExternalInput").ap()
    W = nc.dram_tensor("W", [DIN, E], bf16, kind="ExternalInput").ap()
    b = nc.dram_tensor("b", [E, 1], f32, kind="ExternalInput").ap()
    eye = nc.dram_tensor("eye", [P, P], f32, kind="ExternalInput").ap()
    outT = nc.dram_tensor("outT", [E, B], f32, kind="ExternalOutput").ap()

    dbg_aps = None
    if DEBUG_DUMPS:
        dbg_aps = (
            nc.dram_tensor("dbg_sq", [P, NT], f32, kind="ExternalOutput").ap(),
            nc.dram_tensor("dbg_r", [P, NT], f32, kind="ExternalOutput").ap(),
            nc.dram_tensor("dbg_s", [P, NT], f32, kind="ExternalOutput").ap(),
            nc.dram_tensor("dbg_a0", [P, B], bf16, kind="ExternalOutput").ap(),
            nc.dram_tensor("dbg_a2", [P, B], bf16, kind="ExternalOutput").ap(),
            nc.dram_tensor("dbg_xe", [P, NT, E], bf16,
                           kind="ExternalOutput").ap(),
            nc.dram_tensor("dbg_augL", [AUG, N], bf16,
                           kind="ExternalOutput").ap(),
            nc.dram_tensor("dbg_augR", [AUG, B], bf16,
                           kind="ExternalOutput").ap(),
        )

    with tile.TileContext(nc) as tc:
        build_body(tc, outT, xT, xTl, W, b, eye, dbg_aps)
    nc.compile()
    return nc


def make_in_maps(x, W_emb, b_emb):
    import ml_dtypes

    bf = ml_dtypes.bfloat16
    xT = np.ascontiguousarray(x.T).astype(bf)
    eye = np.eye(P, dtype=np.float32)
    bb = np.asarray(b_emb, dtype=np.float32).reshape(E, 1)
    Wf = np.asarray(W_emb, dtype=np.float32).astype(bf)
    in_maps = []
    for c in range(C):
        in_maps.append({
            "xT": xT,
            "xTl": np.ascontiguousarray(xT[:, c * B:(c + 1) * B]),
            "W": Wf,
            "b": bb,
            "eye": eye,
        })
    return in_maps


def kernel(x, W_emb, b_emb, _trace=False, _tmpdir=None):
    _import_concourse()
    from concourse import bass_utils

    key = "nc"
    if key not in _NC_CACHE:
        _NC_CACHE[key] = _build_nc()
    nc = _NC_CACHE[key]

    in_maps = make_in_maps(np.asarray(x), np.asarray(W_emb), np.asarray(b_emb))
    res = bass_utils.run_bass_kernel_spmd(
        nc, in_maps, core_ids=list(range(C)),
        trace=_trace, tmpdir=_tmpdir,
    )
    blocks = [np.asarray(res.results[c]["outT"]) for c in range(C)]
    outT = np.concatenate(blocks, axis=1)          # [E, N]
    out = np.ascontiguousarray(outT.T).astype(np.float32)  # [N, E]
    if _trace:
        return out, res
    return out
